# revision 35
# baseline (speedup 1.0000x reference)
"""Trainium2 Bass kernel for MultiHeadAttention with ALiBi + causal mask.

Problem: B=2, T=2048, E=1024, H=16, D=64. Returns (out, attn) like the
reference:
    Q/K/V = einsum('bte,hed->bhtd', x, W{q,k,v})
    scores = QK^T/sqrt(D) + alibi_bias (causal)
    attn   = softmax(scores)
    out    = concat_heads(attn @ V) @ Wo^T + bo

Sharding: 2 heads per core across 8 cores (tensor parallel). Each core
computes its heads' QKV + attention + its slice of the output projection;
the output-projection partial sums are reduced on the host.

Device-side design (per core):
  - All matmuls in bf16 (fp32 PSUM accumulation). bf16 enables FWL fast
    weight loads and pipelined LDWEIGHTS; fp32/fp32r matmuls serialize
    LDWEIGHTS+MATMUL (~3x slower, measured on HW).
  - x is pre-transposed on host to x^T [B, E, T] (bf16) so the embedding
    dim sits on SBUF partitions for the QKV projections.
  - Q^T/K^T/V^T produced in [d, t] layout directly.
  - scores computed TRANSPOSED: S^T[j, i] = Ka.T @ Qa with the ALiBi bias
    folded into 8 augmentation rows of the contraction, split so every
    component is exactly representable in bf16:
       slope*j = 64*A + B + fh + fl   (A,B ints; fh=bf16(frac), fl=resid)
    rows (x8 pre-scale since exp applies 1/8): [512A, 8B, 8fh, 8fl, 1,1,1,1]
    paired with [1,1,1,1, -512A', -8B', -8fh', -8fl'].
  - softmax: exp on ACT (scale=1/8) -> bf16 P tiles; causal mask via
    gpsimd affine_select (fill 0); row sums come FREE from an extra ones
    column in the V operand of the PV matmul (accumulate in PSUM row 64).
  - attn is written to HBM as bf16, transposed attn_T[b, h, j, i]; the
    host upcasts (exact widening) and returns a transposed view. The
    upper-triangle tiles are never written (zero-initialized outputs).
  - out^T[e, t] partials written bf16 per core; host sums + bias.
"""

import numpy as np

B, T, E, H = 2, 2048, 1024, 16
D = E // H  # 64
NCORES = 8
H_LOC = H // NCORES  # 2 heads per core
KT = E // 128  # 8 k-tiles for the QKV projections
TI = 512  # i-block width (scores free dim / PSUM bank)
TJ = 128  # j-tile width (scores partition dim)
NIB = T // TI  # 4
NJT = T // TJ  # 16
NAUG = 8  # alibi augmentation rows per head (per side)

_CACHE = {}


def _bf16(a):
    import ml_dtypes

    return np.asarray(a, dtype=ml_dtypes.bfloat16)


def _f32_from_bf16(a):
    # exact widening via bit trick (fast, avoids ml_dtypes cast loops)
    u = np.asarray(a).view(np.uint16).astype(np.uint32) << 16
    return u.view(np.float32)


def _host_inputs(x, Wq, Wk, Wv, Wo):
    """Build the per-core input maps (numpy only)."""
    x = np.ascontiguousarray(np.asarray(x, dtype=np.float32))
    xT = np.ascontiguousarray(x.transpose(0, 2, 1))  # [B, E, T]
    xT16 = _bf16(xT)

    base = 2.0 ** (-8.0 / H)
    j = np.arange(T, dtype=np.float64)

    in_maps = []
    for c in range(NCORES):
        h0, h1 = H_LOC * c, H_LOC * c + 1
        wq = np.concatenate([Wq[h0], Wq[h1]], axis=1)  # [E, 128]
        wk = np.concatenate([Wk[h0], Wk[h1]], axis=1)
        wv = np.concatenate([Wv[h0], Wv[h1]], axis=1)
        wot = Wo[:, 128 * c : 128 * (c + 1)].T  # [128(k), E]

        # ALiBi augmentation rows (see module docstring).
        aug = np.zeros((2 * NAUG * H_LOC, T), dtype=np.float64)
        for hl in range(H_LOC):
            slope = base ** (H_LOC * c + hl + 1)
            sj = slope * j
            A = np.floor(sj / 64.0)
            Bp = np.floor(sj) - 64.0 * A
            frac = sj - np.floor(sj)
            fh = _f32_from_bf16(_bf16(frac)).astype(np.float64)
            fl = frac - fh
            o = 2 * NAUG * hl
            # K-side rows
            aug[o + 0] = 512.0 * A
            aug[o + 1] = 8.0 * Bp
            aug[o + 2] = 8.0 * fh
            aug[o + 3] = 8.0 * fl
            aug[o + 4 : o + 8] = 1.0
            # Q-side rows
            aug[o + 8 : o + 12] = 1.0
            aug[o + 12] = -512.0 * A
            aug[o + 13] = -8.0 * Bp
            aug[o + 14] = -8.0 * fh
            aug[o + 15] = -8.0 * fl
        in_maps.append(
            {
                "xT": xT16,
                "wq": _bf16(wq),
                "wk": _bf16(wk),
                "wv": _bf16(wv),
                "wot": _bf16(wot),
                "aug": _bf16(aug),
            }
        )
    return in_maps


def _build_program():
    import concourse.bass as bass
    import concourse.tile as tile
    from concourse import bacc, mybir
    from concourse.bass import ts
    from concourse.masks import make_identity

    f32 = mybir.dt.float32
    bf16 = mybir.dt.bfloat16
    KA = 64 + NAUG  # contraction rows for the scores matmul

    nc = bacc.Bacc("TRN2", target_bir_lowering=False, debug=False)

    xT_d = nc.dram_tensor("xT", [B, E, T], bf16, kind="ExternalInput").ap()
    wq_d = nc.dram_tensor("wq", [E, 128], bf16, kind="ExternalInput").ap()
    wk_d = nc.dram_tensor("wk", [E, 128], bf16, kind="ExternalInput").ap()
    wv_d = nc.dram_tensor("wv", [E, 128], bf16, kind="ExternalInput").ap()
    wot_d = nc.dram_tensor("wot", [128, E], bf16, kind="ExternalInput").ap()
    aug_d = nc.dram_tensor(
        "aug", [2 * NAUG * H_LOC, T], bf16, kind="ExternalInput"
    ).ap()

    attnT_d = nc.dram_tensor(
        "attnT", [B, H_LOC, T, T], bf16, kind="ExternalOutput"
    ).ap()
    outT_d = nc.dram_tensor("outT", [B, E, T], bf16, kind="ExternalOutput").ap()

    with tile.TileContext(nc) as tc:
        with (
            tc.tile_pool(name="const", bufs=1) as const_pool,
            tc.tile_pool(name="qkvT", bufs=1) as qkvT_pool,
        ):
            # ---- constants ----
            ident = const_pool.tile([128, 128], bf16)
            make_identity(nc, ident[:])
            # additive causal masks (0 valid / -1e9 masked) for the 4
            # diagonal-crossing tile offsets, added to scores pre-exp so
            # exp() gives exact zeros and can never produce inf.
            masks = []
            for k in range(4):
                mk = const_pool.tile([128, TI], f32, tag=f"mask{k}", name=f"mask{k}")
                nc.gpsimd.memset(mk[:], 0.0)
                nc.gpsimd.affine_select(
                    out=mk[:],
                    in_=mk[:],
                    compare_op=mybir.AluOpType.is_ge,
                    fill=-1.0e9,
                    base=-TJ * k,
                    pattern=[[1, TI]],
                    channel_multiplier=-1,
                )
                masks.append(mk)
            wot_s = const_pool.tile([128, E], bf16)
            nc.sync.dma_start(wot_s[:], wot_d)

            # ---- phase A: QKV projections (transposed outputs, bf16) ----
            QTt = qkvT_pool.tile([128, B * T], bf16, tag="QTt")
            KTt = qkvT_pool.tile([128, B * T], bf16, tag="KTt")
            VTt = qkvT_pool.tile([128, B * T], bf16, tag="VTt")
            proj_out = {"q": QTt, "k": KTt, "v": VTt}

            w_pool = tc.alloc_tile_pool(name="wqkv", bufs=1)
            xt_pool = tc.alloc_tile_pool(name="xt", bufs=8)
            w_tiles = {}
            for name, dram in (("q", wq_d), ("k", wk_d), ("v", wv_d)):
                w = w_pool.tile(
                    [128, KT, 128], bf16, tag=f"w{name}", name=f"w{name}"
                )
                nc.scalar.dma_start(
                    w[:], dram.rearrange("(kt p) m -> p kt m", p=128)
                )
                w_tiles[name] = w

            def emit_qkv(b, qkv_psum_pool):
                xts = []
                for kt in range(KT):
                    xt = xt_pool.tile([128, T], bf16, tag="xt")
                    nc.scalar.dma_start(xt[:], xT_d[b, ts(kt, 128), :])
                    xts.append(xt)
                for nb in range(T // TI):
                    for name in ("q", "k", "v"):
                        ps = qkv_psum_pool.tile([128, TI], f32, tag="sps")
                        for kt in range(KT):
                            nc.tensor.matmul(
                                ps[:],
                                w_tiles[name][:, kt, :],
                                xts[kt][:, ts(nb, TI)],
                                start=(kt == 0),
                                stop=(kt == KT - 1),
                            )
                        nc.vector.tensor_copy(
                            proj_out[name][
                                :, b * T + nb * TI : b * T + (nb + 1) * TI
                            ],
                            ps[:],
                        )

            # ---- phases A2 + B + C ----
            with (
                tc.tile_pool(name="vj", bufs=4) as vj_pool,
                tc.tile_pool(name="qa", bufs=4) as qa_pool,
                tc.tile_pool(name="ka", bufs=4) as ka_pool,
                tc.tile_pool(name="pbuf", bufs=3) as p_pool,
                tc.tile_pool(name="small", bufs=3) as small_pool,
                tc.tile_pool(name="ct", bufs=2) as ct_pool,
                tc.tile_pool(name="outp", bufs=4) as out_pool,
                tc.tile_pool(name="ps_s", bufs=5, space="PSUM") as ps_s,
                tc.tile_pool(name="ps_c", bufs=2, space="PSUM") as ps_c,
                tc.tile_pool(name="ps_x", bufs=1, space="PSUM") as ps_x,
            ):
                def emit_build(b, h):
                    o = 2 * NAUG * h
                    vj = vj_pool.tile([128, NJT, 65], bf16, tag="vj", name=f"vj{b}{h}")
                    nc.vector.memset(vj[:, :, 64:65], 1.0)
                    for jt in range(NJT):
                        pvt = ps_x.tile([128, 512], f32, tag="x", name="pvt").bitcast(bf16)[:, 0:64]
                        nc.tensor.transpose(
                            pvt[:],
                            VTt[ts(h, 64), b * T + jt * TJ : b * T + (jt + 1) * TJ],
                            ident[ts(h, 64), ts(h, 64)],
                        )
                        nc.vector.tensor_copy(vj[:, jt, 0:64], pvt[:])
                    qa = qa_pool.tile([KA, T], bf16, tag="qa", name=f"qa{b}{h}")
                    ka = ka_pool.tile([KA, T], bf16, tag="ka", name=f"ka{b}{h}")
                    if h == 0:
                        nc.vector.tensor_copy(qa[0:64, :], QTt[0:64, ts(b, T)])
                        nc.vector.tensor_copy(ka[0:64, :], KTt[0:64, ts(b, T)])
                    else:
                        nc.sync.dma_start(qa[0:64, :], QTt[64:128, ts(b, T)])
                        nc.sync.dma_start(ka[0:64, :], KTt[64:128, ts(b, T)])
                    nc.sync.dma_start(ka[64:KA, :], aug_d[o : o + NAUG, :])
                    nc.sync.dma_start(
                        qa[64:KA, :], aug_d[o + NAUG : o + 2 * NAUG, :]
                    )
                    vjs[b, h], qas[b, h], kas[b, h] = vj, qa, ka

                vjs, qas, kas = {}, {}, {}

                # software-pipelined: the 16 normalize TTs + attn DMA of
                # block (b,h,ib) are emitted AFTER the NEXT block's scores/
                # exp section (across h/b boundaries too), so the DVE stream
                # never head-of-line-blocks on the rowsum broadcast.
                pending = None

                def flush_pending():
                    nonlocal pending
                    if pending is None:
                        return
                    fb, fh, fib, fnjt, fp, fbc = pending
                    for jt in range(fnjt):
                        eng = nc.vector if jt % 2 == 0 else nc.gpsimd
                        eng.tensor_tensor(
                            out=fp[:, jt, :],
                            in0=fp[:, jt, :],
                            in1=fbc[:],
                            op=mybir.AluOpType.mult,
                        )
                    nc.sync.dma_start(
                        attnT_d[fb, fh, 0 : fnjt * TJ, ts(fib, TI)]
                        .rearrange("(jt p) f -> p jt f", p=TJ),
                        fp[:, 0:fnjt, :],
                    )
                    pending = None

                emit_qkv(0, ps_s)
                emit_build(0, 0)
                emit_build(0, 1)
                for b in range(B):
                    # combined normalized ctx for this b: rows 0-63 = h0,
                    # rows 64-127 = h1 (h1 arrives via small s2s DMAs)
                    ctc = ct_pool.tile([128, T], bf16, tag="ctc", name="ctc")
                    ct1 = ct_pool.tile([64, T], bf16, tag="ct1", name="ct1")
                    for h in range(H_LOC):
                        if b == 0 and h == 1:
                            # b=1's QKV + builds: emitted mid-attention so the
                            # scheduler fills b0-attention PE gaps with them
                            emit_qkv(1, ps_s)
                            emit_build(1, 0)
                            emit_build(1, 1)
                        vj, qa, ka = vjs[b, h], qas[b, h], kas[b, h]
                        # -- attention per i-block --
                        for ib in range(NIB):
                            n_jt = 4 * (ib + 1)
                            cps = ps_c.tile([65, TI], f32, tag="cps")
                            pbig = p_pool.tile([128, NJT, TI], bf16, tag="p")
                            for jt in range(n_jt):
                                sps = ps_s.tile([128, TI], f32, tag="sps")
                                nc.tensor.matmul(
                                    sps[:],
                                    ka[:, ts(jt, TJ)],
                                    qa[:, ts(ib, TI)],
                                    start=True,
                                    stop=True,
                                )
                                if jt >= 4 * ib:  # diagonal-crossing tile
                                    nc.vector.tensor_tensor(
                                        out=sps[:],
                                        in0=sps[:],
                                        in1=masks[jt - 4 * ib][:],
                                        op=mybir.AluOpType.add,
                                    )
                                nc.scalar.activation(
                                    pbig[:, jt, :],
                                    sps[:],
                                    mybir.ActivationFunctionType.Exp,
                                    scale=0.125,
                                )
                            # previous block's normalize + attn write go here
                            flush_pending()
                            # PV burst
                            for jt in range(n_jt):
                                nc.tensor.matmul(
                                    cps[:],
                                    vj[:, jt, :],
                                    pbig[:, jt, :],
                                    start=(jt == 0),
                                    stop=(jt == n_jt - 1),
                                )

                            # row sums in cps row 64. Reshape to [128,4] so
                            # the iterative-divide reciprocal runs on all 128
                            # lanes (FD=4) instead of one lane at FD=512; then
                            # reshape back to [1,512] on partition 0 for the
                            # gpsimd broadcast (which reads physical part 0).
                            rs = small_pool.tile([65, TI], f32, tag="rs")
                            nc.vector.tensor_copy(rs[64:65, :], cps[64:65, :])
                            rec4 = small_pool.tile([128, 4], f32, tag="rec4")
                            nc.sync.dma_start(rec4[:], rs[64:65, :])
                            nc.vector.reciprocal(rec4[:], rec4[:])
                            rec = small_pool.tile([1, TI], f32, tag="rec")
                            nc.sync.dma_start(rec[0:1, :], rec4[:])
                            rec16 = small_pool.tile([1, TI], bf16, tag="rec16")
                            nc.vector.tensor_copy(rec16[0:1, :], rec[0:1, :])
                            bc = small_pool.tile([128, TI], bf16, tag="bc")
                            nc.gpsimd.partition_broadcast(bc[:], rec16[0:1, :])
                            bc32 = small_pool.tile([64, TI], f32, tag="bc32")
                            nc.gpsimd.partition_broadcast(bc32[:], rec[0:1, :])

                            # normalized ctx slice (prompt: frees cps)
                            dst = (
                                ctc[0:64, ts(ib, TI)]
                                if h == 0
                                else ct1[:, ts(ib, TI)]
                            )
                            nc.vector.tensor_tensor(
                                out=dst,
                                in0=cps[0:64, :],
                                in1=bc32[:],
                                op=mybir.AluOpType.mult,
                            )
                            if h == 1:
                                nc.sync.dma_start(
                                    ctc[64:128, ts(ib, TI)], ct1[:, ts(ib, TI)]
                                )
                            pending = (b, h, ib, n_jt, pbig, bc)

                    # ---- phase C: output projection for this b (ib-outer so
                    # finished i-blocks flush to HBM immediately) ----
                    for ib in range(NIB):
                        for et in range(E // 128):
                            ops = ps_x.tile([128, 512], f32, tag="x", name="ops")
                            nc.tensor.matmul(
                                ops[:],
                                wot_s[:, ts(et, 128)],
                                ctc[:, ts(ib, TI)],
                                start=True,
                                stop=True,
                            )
                            ot = out_pool.tile([128, TI], bf16, tag="ot")
                            nc.scalar.copy(ot[:], ops[:])
                            nc.sync.dma_start(
                                outT_d[b, ts(et, 128), ts(ib, TI)], ot[:]
                            )

                flush_pending()

            xt_pool.release()
            w_pool.release()

    nc.compile()
    return nc


def _get_program():
    if "nc" not in _CACHE:
        _CACHE["nc"] = _build_program()
    return _CACHE["nc"]


def kernel(x, Wq, Wk, Wv, Wo, bo, _trace=False):
    import concourse.bass_utils as bass_utils

    x = np.asarray(x, dtype=np.float32)
    Wq = np.asarray(Wq, dtype=np.float32)
    Wk = np.asarray(Wk, dtype=np.float32)
    Wv = np.asarray(Wv, dtype=np.float32)
    Wo = np.asarray(Wo, dtype=np.float32)
    bo = np.asarray(bo, dtype=np.float32)

    nc = _get_program()
    in_maps = _host_inputs(x, Wq, Wk, Wv, Wo)
    res = bass_utils.run_bass_kernel_spmd(
        nc, in_maps, core_ids=list(range(NCORES)), trace=_trace
    )
    results = res.results

    # ---- host-side gather / unshard ----
    outT = np.zeros((B, E, T), dtype=np.float32)
    for c in range(NCORES):
        outT += _f32_from_bf16(results[c]["outT"])
    outT += bo[None, :, None]
    out = outT.transpose(0, 2, 1)  # [B, T, E] view

    attnT = np.empty((B, H, T, T), dtype=np.float32)  # [b, h, j, i]
    for c in range(NCORES):
        a = results[c]["attnT"]  # [B, H_LOC, T, T] bf16
        for hl in range(H_LOC):
            attnT[:, H_LOC * c + hl] = _f32_from_bf16(a[:, hl])
    attn = attnT.transpose(0, 1, 3, 2)  # [b, h, i, j] view

    if _trace:
        _CACHE["last_result"] = res
    return out, attn


# revision 36
# speedup vs baseline: 1.4760x; 1.4760x over previous
"""Trainium2 Bass kernel for MultiHeadAttention with ALiBi + causal mask.

Problem: B=2, T=2048, E=1024, H=16, D=64. Returns (out, attn) like the
reference:
    Q/K/V = einsum('bte,hed->bhtd', x, W{q,k,v})
    scores = QK^T/sqrt(D) + alibi_bias (causal)
    attn   = softmax(scores)
    out    = concat_heads(attn @ V) @ Wo^T + bo

Sharding: 2 heads per core across 8 cores (tensor parallel). Each core
computes its heads' QKV + attention + its slice of the output projection;
the output-projection partial sums are reduced on the host.

Device-side design (per core):
  - All matmuls in bf16 (fp32 PSUM accumulation). bf16 enables FWL fast
    weight loads and pipelined LDWEIGHTS; fp32/fp32r matmuls serialize
    LDWEIGHTS+MATMUL (~3x slower, measured on HW).
  - x is pre-transposed on host to x^T [B, E, T] (bf16) so the embedding
    dim sits on SBUF partitions for the QKV projections.
  - Q^T/K^T/V^T produced in [d, t] layout directly.
  - scores computed TRANSPOSED: S^T[j, i] = Ka.T @ Qa with the ALiBi bias
    folded into 8 augmentation rows of the contraction, split so every
    component is exactly representable in bf16:
       slope*j = 64*A + B + fh + fl   (A,B ints; fh=bf16(frac), fl=resid)
    rows (x8 pre-scale since exp applies 1/8): [512A, 8B, 8fh, 8fl, 1,1,1,1]
    paired with [1,1,1,1, -512A', -8B', -8fh', -8fl'].
  - softmax: exp on ACT (scale=1/8) -> bf16 P tiles; causal mask via
    gpsimd affine_select (fill 0); row sums come FREE from an extra ones
    column in the V operand of the PV matmul (accumulate in PSUM row 64).
  - attn is written to HBM as bf16, transposed attn_T[b, h, j, i]; the
    host upcasts (exact widening) and returns a transposed view. The
    upper-triangle tiles are never written (zero-initialized outputs).
  - out^T[e, t] partials written bf16 per core; host sums + bias.
"""

import numpy as np

B, T, E, H = 2, 2048, 1024, 16
D = E // H  # 64
NCORES = 8
H_LOC = H // NCORES  # 2 heads per core
KT = E // 128  # 8 k-tiles for the QKV projections
TI = 512  # i-block width (scores free dim / PSUM bank)
TJ = 128  # j-tile width (scores partition dim)
NIB = T // TI  # 4
NJT = T // TJ  # 16
NAUG = 8  # alibi augmentation rows per head (per side)

_CACHE = {}


def _bf16(a):
    import ml_dtypes

    return np.asarray(a, dtype=ml_dtypes.bfloat16)


def _f32_from_bf16(a):
    # exact widening via bit trick (fast, avoids ml_dtypes cast loops)
    u = np.asarray(a).view(np.uint16).astype(np.uint32) << 16
    return u.view(np.float32)


def _host_inputs(x, Wq, Wk, Wv, Wo):
    """Build the per-core input maps (numpy only)."""
    x = np.ascontiguousarray(np.asarray(x, dtype=np.float32))
    xT = np.ascontiguousarray(x.transpose(0, 2, 1))  # [B, E, T]
    xT16 = _bf16(xT)

    base = 2.0 ** (-8.0 / H)
    j = np.arange(T, dtype=np.float64)

    in_maps = []
    for c in range(NCORES):
        h0, h1 = H_LOC * c, H_LOC * c + 1
        wq = np.concatenate([Wq[h0], Wq[h1]], axis=1)  # [E, 128]
        wk = np.concatenate([Wk[h0], Wk[h1]], axis=1)
        wv = np.concatenate([Wv[h0], Wv[h1]], axis=1)
        wot = Wo[:, 128 * c : 128 * (c + 1)].T  # [128(k), E]

        # ALiBi augmentation rows (see module docstring).
        aug = np.zeros((2 * NAUG * H_LOC, T), dtype=np.float64)
        for hl in range(H_LOC):
            slope = base ** (H_LOC * c + hl + 1)
            sj = slope * j
            A = np.floor(sj / 64.0)
            Bp = np.floor(sj) - 64.0 * A
            frac = sj - np.floor(sj)
            fh = _f32_from_bf16(_bf16(frac)).astype(np.float64)
            fl = frac - fh
            o = 2 * NAUG * hl
            # K-side rows
            aug[o + 0] = 512.0 * A
            aug[o + 1] = 8.0 * Bp
            aug[o + 2] = 8.0 * fh
            aug[o + 3] = 8.0 * fl
            aug[o + 4 : o + 8] = 1.0
            # Q-side rows
            aug[o + 8 : o + 12] = 1.0
            aug[o + 12] = -512.0 * A
            aug[o + 13] = -8.0 * Bp
            aug[o + 14] = -8.0 * fh
            aug[o + 15] = -8.0 * fl
        in_maps.append(
            {
                "xT": xT16,
                "wq": _bf16(wq),
                "wk": _bf16(wk),
                "wv": _bf16(wv),
                "wot": _bf16(wot),
                "aug": _bf16(aug),
            }
        )
    return in_maps


def _build_program():
    import concourse.bass as bass
    import concourse.tile as tile
    from concourse import bacc, mybir
    from concourse.bass import ts
    from concourse.masks import make_identity

    f32 = mybir.dt.float32
    bf16 = mybir.dt.bfloat16
    KA = 64 + NAUG  # contraction rows for the scores matmul

    nc = bacc.Bacc("TRN2", target_bir_lowering=False, debug=False)

    xT_d = nc.dram_tensor("xT", [B, E, T], bf16, kind="ExternalInput").ap()
    wq_d = nc.dram_tensor("wq", [E, 128], bf16, kind="ExternalInput").ap()
    wk_d = nc.dram_tensor("wk", [E, 128], bf16, kind="ExternalInput").ap()
    wv_d = nc.dram_tensor("wv", [E, 128], bf16, kind="ExternalInput").ap()
    wot_d = nc.dram_tensor("wot", [128, E], bf16, kind="ExternalInput").ap()
    aug_d = nc.dram_tensor(
        "aug", [2 * NAUG * H_LOC, T], bf16, kind="ExternalInput"
    ).ap()

    attnT_d = nc.dram_tensor(
        "attnT", [B, H_LOC, T, T], bf16, kind="ExternalOutput"
    ).ap()
    outT_d = nc.dram_tensor("outT", [B, E, T], bf16, kind="ExternalOutput").ap()

    with tile.TileContext(nc) as tc:
        with (
            tc.tile_pool(name="const", bufs=1) as const_pool,
            tc.tile_pool(name="qkvT", bufs=1) as qkvT_pool,
        ):
            # ---- constants ----
            ident = const_pool.tile([128, 128], bf16)
            make_identity(nc, ident[:])
            # additive causal masks (0 valid / -1e9 masked) for the 4
            # diagonal-crossing tile offsets, added to scores pre-exp so
            # exp() gives exact zeros and can never produce inf.
            masks = []
            for k in range(4):
                mk = const_pool.tile([128, TI], f32, tag=f"mask{k}", name=f"mask{k}")
                nc.gpsimd.memset(mk[:], 0.0)
                nc.gpsimd.affine_select(
                    out=mk[:],
                    in_=mk[:],
                    compare_op=mybir.AluOpType.is_ge,
                    fill=-1.0e9,
                    base=-TJ * k,
                    pattern=[[1, TI]],
                    channel_multiplier=-1,
                )
                masks.append(mk)
            wot_s = const_pool.tile([128, E], bf16)
            nc.sync.dma_start(wot_s[:], wot_d)

            # ---- phase A: QKV projections (transposed outputs, bf16) ----
            QTt = qkvT_pool.tile([128, B * T], bf16, tag="QTt")
            KTt = qkvT_pool.tile([128, B * T], bf16, tag="KTt")
            VTt = qkvT_pool.tile([128, B * T], bf16, tag="VTt")
            proj_out = {"q": QTt, "k": KTt, "v": VTt}

            w_pool = tc.alloc_tile_pool(name="wqkv", bufs=1)
            xt_pool = tc.alloc_tile_pool(name="xt", bufs=8)
            w_tiles = {}
            for name, dram in (("q", wq_d), ("k", wk_d), ("v", wv_d)):
                w = w_pool.tile(
                    [128, KT, 128], bf16, tag=f"w{name}", name=f"w{name}"
                )
                nc.scalar.dma_start(
                    w[:], dram.rearrange("(kt p) m -> p kt m", p=128)
                )
                w_tiles[name] = w

            def emit_qkv(b, qkv_psum_pool):
                xts = []
                for kt in range(KT):
                    xt = xt_pool.tile([128, T], bf16, tag="xt")
                    nc.scalar.dma_start(xt[:], xT_d[b, ts(kt, 128), :])
                    xts.append(xt)
                for nbp in range(T // TI // 2):
                    ps = qkv_psum_pool.tile([128, 2, TI], f32, tag="sps")
                    for name_i, name in enumerate(("q", "k", "v")):
                        for k in range(2):
                            nb = 2 * nbp + k
                            for kt in range(KT):
                                nc.tensor.matmul(
                                    ps[:, k, :],
                                    w_tiles[name][:, kt, :],
                                    xts[kt][:, ts(nb, TI)],
                                    start=(kt == 0),
                                    stop=(kt == KT - 1),
                                )
                        nc.vector.tensor_copy(
                            proj_out[name][
                                :, b * T + 2 * nbp * TI : b * T + 2 * (nbp + 1) * TI
                            ],
                            ps[:].rearrange("p a f -> p (a f)"),
                        )
                        if name_i < 2:
                            ps = qkv_psum_pool.tile([128, 2, TI], f32, tag="sps")

            # ---- phases A2 + B + C ----
            with (
                tc.tile_pool(name="vj", bufs=4) as vj_pool,
                tc.tile_pool(name="qa", bufs=4) as qa_pool,
                tc.tile_pool(name="ka", bufs=4) as ka_pool,
                tc.tile_pool(name="pbuf", bufs=3) as p_pool,
                tc.tile_pool(name="small", bufs=3) as small_pool,
                tc.tile_pool(name="ct", bufs=2) as ct_pool,
                tc.tile_pool(name="outp", bufs=4) as out_pool,
                tc.tile_pool(name="ps_s", bufs=2, space="PSUM") as ps_s,
                tc.tile_pool(name="ps_c", bufs=2, space="PSUM") as ps_c,
                tc.tile_pool(name="ps_x", bufs=2, space="PSUM") as ps_x,
            ):
                def emit_build(b, h):
                    o = 2 * NAUG * h
                    vj = vj_pool.tile([128, NJT, 65], bf16, tag="vj", name=f"vj{b}{h}")
                    nc.vector.memset(vj[:, :, 64:65], 1.0)
                    for jt in range(NJT):
                        pvt = ps_x.tile([128, 512], f32, tag="x", name="pvt").bitcast(bf16)[:, 0:64]
                        nc.tensor.transpose(
                            pvt[:],
                            VTt[ts(h, 64), b * T + jt * TJ : b * T + (jt + 1) * TJ],
                            ident[ts(h, 64), ts(h, 64)],
                        )
                        nc.vector.tensor_copy(vj[:, jt, 0:64], pvt[:])
                    qa = qa_pool.tile([KA, T], bf16, tag="qa", name=f"qa{b}{h}")
                    ka = ka_pool.tile([KA, T], bf16, tag="ka", name=f"ka{b}{h}")
                    if h == 0:
                        nc.vector.tensor_copy(qa[0:64, :], QTt[0:64, ts(b, T)])
                        nc.vector.tensor_copy(ka[0:64, :], KTt[0:64, ts(b, T)])
                    else:
                        nc.sync.dma_start(qa[0:64, :], QTt[64:128, ts(b, T)])
                        nc.sync.dma_start(ka[0:64, :], KTt[64:128, ts(b, T)])
                    nc.sync.dma_start(ka[64:KA, :], aug_d[o : o + NAUG, :])
                    nc.sync.dma_start(
                        qa[64:KA, :], aug_d[o + NAUG : o + 2 * NAUG, :]
                    )
                    vjs[b, h], qas[b, h], kas[b, h] = vj, qa, ka

                vjs, qas, kas = {}, {}, {}

                # software-pipelined: the 16 normalize TTs + attn DMA of
                # block (b,h,ib) are emitted AFTER the NEXT block's scores/
                # exp section (across h/b boundaries too), so the DVE stream
                # never head-of-line-blocks on the rowsum broadcast.
                pending = None

                def flush_pending():
                    nonlocal pending
                    if pending is None:
                        return
                    fb, fh, fib, fnjt, fp, fbc = pending
                    for jt in range(fnjt):
                        nc.vector.tensor_tensor(
                            out=fp[:, jt, :],
                            in0=fp[:, jt, :],
                            in1=fbc[:],
                            op=mybir.AluOpType.mult,
                        )
                    nc.sync.dma_start(
                        attnT_d[fb, fh, 0 : fnjt * TJ, ts(fib, TI)]
                        .rearrange("(jt p) f -> p jt f", p=TJ),
                        fp[:, 0:fnjt, :],
                    )
                    pending = None

                emit_qkv(0, ps_s)
                emit_build(0, 0)
                emit_build(0, 1)
                for b in range(B):
                    # combined normalized ctx for this b: rows 0-63 = h0,
                    # rows 64-127 = h1 (h1 arrives via small s2s DMAs)
                    ctc = ct_pool.tile([128, T], bf16, tag="ctc", name="ctc")
                    ct1 = ct_pool.tile([64, T], bf16, tag="ct1", name="ct1")
                    for h in range(H_LOC):
                        if b == 0 and h == 1:
                            # b=1's QKV + builds: emitted mid-attention so the
                            # scheduler fills b0-attention PE gaps with them
                            emit_qkv(1, ps_s)
                            emit_build(1, 0)
                            emit_build(1, 1)
                        vj, qa, ka = vjs[b, h], qas[b, h], kas[b, h]
                        # -- attention per i-block --
                        for ib in range(NIB):
                            n_jt = 4 * (ib + 1)
                            cps = ps_c.tile([65, TI], f32, tag="cps")
                            pbig = p_pool.tile([128, NJT, TI], bf16, tag="p")
                            for jp in range(n_jt // 2):
                                sps = ps_s.tile([128, 2, TI], f32, tag="sps")
                                for k in range(2):
                                    jt = 2 * jp + k
                                    nc.tensor.matmul(
                                        sps[:, k, :],
                                        ka[:, ts(jt, TJ)],
                                        qa[:, ts(ib, TI)],
                                        start=True,
                                        stop=True,
                                    )
                                    if jt >= 4 * ib:  # diagonal-crossing tile
                                        nc.vector.tensor_tensor(
                                            out=sps[:, k, :],
                                            in0=sps[:, k, :],
                                            in1=masks[jt - 4 * ib][:],
                                            op=mybir.AluOpType.add,
                                        )
                                nc.scalar.activation(
                                    pbig[:, 2 * jp : 2 * jp + 2, :],
                                    sps[:],
                                    mybir.ActivationFunctionType.Exp,
                                    scale=0.125,
                                )
                            # previous block's normalize + attn write go here
                            flush_pending()
                            # PV burst
                            for jt in range(n_jt):
                                nc.tensor.matmul(
                                    cps[:],
                                    vj[:, jt, :],
                                    pbig[:, jt, :],
                                    start=(jt == 0),
                                    stop=(jt == n_jt - 1),
                                )

                            # row sums in cps row 64. Reshape to [128,4] so
                            # the iterative-divide reciprocal runs on all 128
                            # lanes (FD=4) instead of one lane at FD=512; then
                            # reshape back to [1,512] on partition 0 for the
                            # gpsimd broadcast (which reads physical part 0).
                            rs = small_pool.tile([65, TI], f32, tag="rs")
                            nc.vector.tensor_copy(rs[64:65, :], cps[64:65, :])
                            rec4 = small_pool.tile([128, 4], f32, tag="rec4")
                            nc.sync.dma_start(rec4[:], rs[64:65, :])
                            nc.vector.reciprocal(rec4[:], rec4[:])
                            rec = small_pool.tile([1, TI], f32, tag="rec")
                            nc.sync.dma_start(rec[0:1, :], rec4[:])
                            rec16 = small_pool.tile([1, TI], bf16, tag="rec16")
                            nc.vector.tensor_copy(rec16[0:1, :], rec[0:1, :])
                            bc = small_pool.tile([128, TI], bf16, tag="bc")
                            nc.gpsimd.partition_broadcast(bc[:], rec16[0:1, :])
                            bc32 = small_pool.tile([64, TI], f32, tag="bc32")
                            nc.gpsimd.partition_broadcast(bc32[:], rec[0:1, :])

                            # normalized ctx slice (prompt: frees cps)
                            dst = (
                                ctc[0:64, ts(ib, TI)]
                                if h == 0
                                else ct1[:, ts(ib, TI)]
                            )
                            nc.vector.tensor_tensor(
                                out=dst,
                                in0=cps[0:64, :],
                                in1=bc32[:],
                                op=mybir.AluOpType.mult,
                            )
                            if h == 1:
                                nc.sync.dma_start(
                                    ctc[64:128, ts(ib, TI)], ct1[:, ts(ib, TI)]
                                )
                            pending = (b, h, ib, n_jt, pbig, bc)

                    # ---- phase C: output projection for this b (ib-outer so
                    # finished i-blocks flush to HBM immediately) ----
                    for ib in range(NIB):
                        for et in range(E // 128):
                            ops = ps_x.tile([128, 512], f32, tag="x", name="ops")
                            nc.tensor.matmul(
                                ops[:],
                                wot_s[:, ts(et, 128)],
                                ctc[:, ts(ib, TI)],
                                start=True,
                                stop=True,
                            )
                            ot = out_pool.tile([128, TI], bf16, tag="ot")
                            nc.scalar.copy(ot[:], ops[:])
                            nc.sync.dma_start(
                                outT_d[b, ts(et, 128), ts(ib, TI)], ot[:]
                            )

                flush_pending()

            xt_pool.release()
            w_pool.release()

    nc.compile()
    return nc


def _get_program():
    if "nc" not in _CACHE:
        _CACHE["nc"] = _build_program()
    return _CACHE["nc"]


def kernel(x, Wq, Wk, Wv, Wo, bo, _trace=False):
    import concourse.bass_utils as bass_utils

    x = np.asarray(x, dtype=np.float32)
    Wq = np.asarray(Wq, dtype=np.float32)
    Wk = np.asarray(Wk, dtype=np.float32)
    Wv = np.asarray(Wv, dtype=np.float32)
    Wo = np.asarray(Wo, dtype=np.float32)
    bo = np.asarray(bo, dtype=np.float32)

    nc = _get_program()
    in_maps = _host_inputs(x, Wq, Wk, Wv, Wo)
    res = bass_utils.run_bass_kernel_spmd(
        nc, in_maps, core_ids=list(range(NCORES)), trace=_trace
    )
    results = res.results

    # ---- host-side gather / unshard ----
    outT = np.zeros((B, E, T), dtype=np.float32)
    for c in range(NCORES):
        outT += _f32_from_bf16(results[c]["outT"])
    outT += bo[None, :, None]
    out = outT.transpose(0, 2, 1)  # [B, T, E] view

    attnT = np.empty((B, H, T, T), dtype=np.float32)  # [b, h, j, i]
    for c in range(NCORES):
        a = results[c]["attnT"]  # [B, H_LOC, T, T] bf16
        for hl in range(H_LOC):
            attnT[:, H_LOC * c + hl] = _f32_from_bf16(a[:, hl])
    attn = attnT.transpose(0, 1, 3, 2)  # [b, h, i, j] view

    if _trace:
        _CACHE["last_result"] = res
    return out, attn
